# revision 37
# baseline (speedup 1.0000x reference)
"""Trainium2 Bass kernel for ragged subword mean pooling (nn_Bert).

Problem: out[b, j] = mean(bert_embedding[b, st_j:ed_j]) if (mask & ed>st) else 0
Shapes: bert_embedding [32, 1024, 768] f32, x_bert_offset [32, 768, 2] i32,
        x_mask [32, 768] i32 -> out [32, 768, 768] f32.

Strategy (pure data parallel, 4 batch rows per core on 8 cores). The kernel is
memory-bound; both HBM streams are pushed to ~1 byte/element:

  - E ships as fp8 e3m4 (4 mantissa bits, ~1.34% elem rms error) in a
    partition-major layout; positions not covered by a valid word are
    compacted away on the host (~2.9 MB/core).
  - The pooling is a one-hot matmul in TRANSPOSED orientation: the e3m4
    E k-tile is the STATIONARY operand ([kp positions, 128 dims] per
    128-dim chunk) and the one-hot A the MOVING operand, so each matmul
    streams only as many rows as there are word-slots in that k-tile.
    Words are grouped BY K-TILE (every word gets one output slot per
    k-tile its span touches; the host sums the partial means), so each
    psum element is written by exactly one matmul: total PE rows =
    6 * slots ~ 13.4k/core vs ~43k for the classic orientation.
  - A[pos, slot] = scale_w/2 at the position's word slot (else 0), where
    scale_w = 127/(QCLIP*sqrt(len)). The e3m4 rounding of the scale is
    EXACTLY cancelled on the host (it dequantizes by the rounded value),
    and /2 keeps all scales inside e3m4 normal range (len=1 -> 15.1).
    A tiles are built on DVE at 4x rate via a u16-packed trick: the
    [128, 128] e3m4 tile is written as [128, 64] u16, out_u16 =
    is_equal(J2, idx_p) * val_p with val_p = e3m4bits(scale/2) << 8*(slot&1).
  - PSUM [128 dims-of-chunk, 6 chunks x nmax_k slots] f32 drains to int8
    with a constant x2 activation scale (undoing the /2), split between
    the scalar and vector engines; out rows pack ~560 slots * 768 dims
    int8 (~1.7 MB/core). Host: i8 / (2*e3m4(scale/2)) summed over a
    word's slots, / len, scattered to the f32 [B, W, D] output.
"""

import sys

if "/opt/trn_rl_repo" not in sys.path:
    sys.path.insert(0, "/opt/trn_rl_repo")

import numpy as np
import ml_dtypes

B, S, W, D = 32, 1024, 768, 768
NCORES = 8
RPC = B // NCORES  # rows per core
KT = S // 128  # max k-tiles (compacted positions)
NCHUNK = D // 128  # 6 dim-chunks

QCLIP = 4.2

_CACHE = {}


def build_program(pairs, repeat=1, io="ext", ehalves=1, ohalves=1,
                  ebufs=4, abufs=18, psbufs=4, obufs=3, avbufs=2,
                  dvek=4, drpat="tail", nomm=False, noout=False, noe=False,
                  outeng="gpsimd", gpsa=0, prebuild=False, drmerge=2,
                  aship=True):
    """Build the SPMD Bass program (one program, run on all 8 cores).

    pairs = (nps, nmax): nps[r] = used positions per row slot (max over
    cores), nmax[r][k] = word-slot count of k-group k (max over cores).
    dvek: how many of each row's trailing k-group drains go to DVE.
    """
    import concourse.tile as tile
    from concourse import bacc, mybir

    nps, nmax = pairs
    nps = list(nps)
    nmax = [list(x) for x in nmax]
    ktr = [(n + 127) // 128 for n in nps]
    slw = [[NCHUNK * x for x in row] for row in nmax]  # osb width per k-group
    sbase = []  # osb column base per (r, k)
    ow = []  # osb width per row slot
    for r in range(RPC):
        c0, bases = 0, []
        for k in range(ktr[r]):
            bases.append(c0)
            c0 += slw[r][k]
        sbase.append(bases)
        ow.append(c0)
    owmax = max(ow)

    f32 = mybir.dt.float32
    f8 = mybir.dt.float8e3
    u16 = mybir.dt.uint16
    i32 = mybir.dt.int32
    i8 = mybir.dt.int8
    AF = mybir.ActivationFunctionType
    OP = mybir.AluOpType

    nc = bacc.Bacc(
        "TRN2", target_bir_lowering=False, debug=False, num_devices=NCORES
    )

    # E in partition-major e3m4 layout: E_in[r, p, k*D+d] = e3m4(E[r, k*128+p, d])
    E_in = nc.dram_tensor("E_in", [RPC, 128, KT * D], f8, kind="ExternalInput").ap()
    # packed per (r, k): column 2*(r*KT+k) = u16-column index of the position's
    # word slot (slot>>1, or -1 if none), column +1 = u16 value pattern
    # (e3m4 bits of scale_w/2, shifted <<8 for odd slots)
    av_in = nc.dram_tensor("av_in", [128, RPC * KT * 2], f32, kind="ExternalInput").ap()
    # pre-built one-hot A tiles (host-side), packed per (r, k)
    A_in = nc.dram_tensor("A_in", [128, RPC * KT * 128], f8, kind="ExternalInput").ap()
    if io == "ext":
        out = nc.dram_tensor("out", [RPC, 128, owmax], i8, kind="ExternalOutput").ap()
        tok = None
    else:
        out = nc.dram_tensor("out_scratch", [RPC, 128, owmax], i8).ap()
        tok = nc.dram_tensor("tok", [128, 16], f32, kind="ExternalOutput").ap()

    with tile.TileContext(nc) as tc:
        with (
            tc.tile_pool(name="const", bufs=1) as cpool,
            tc.tile_pool(name="E", bufs=ebufs) as epool,
            tc.tile_pool(name="bc", bufs=avbufs) as bcpool,
            tc.tile_pool(name="A", bufs=max(abufs, 40) if prebuild else abufs) as apool,
            tc.tile_pool(name="outsb", bufs=obufs) as opool,
            tc.tile_pool(name="psum", bufs=psbufs, space="PSUM") as pspool,
        ):
            # constant column-index tile J2[p, j] = j (u16, compared against
            # the packed slot>>1 index)
            j_i = cpool.tile([128, 64], i32)
            nc.gpsimd.iota(j_i[:], pattern=[[1, 64]], base=0, channel_multiplier=0)
            j2 = cpool.tile([128, 64], u16)
            nc.vector.tensor_copy(j2[:], j_i[:])
            e_const = None
            if noe:
                e_const = cpool.tile([128, KT * D], f8)
                nc.vector.memset(e_const[:], 0.5)

            last_osb = None
            for _ in range(repeat):
                av = abig = None
                if aship:
                    abig = bcpool.tile([128, RPC * KT * 128], f8, tag="Abig")
                    nc.sync.dma_start(abig[:], A_in[:, :])
                else:
                    av = bcpool.tile([128, RPC * KT * 2], f32, tag="av")
                    nc.sync.dma_start(av[:], av_in[:, :])

                # A tiles depend only on av: optionally build them all up
                # front so engine-queue order never blocks PE on drains
                pre = {}
                if prebuild and not aship:
                    for r in range(RPC):
                        for k in range(ktr[r]):
                            c = (r * KT + k) * 2
                            at = apool.tile([128, 128], f8, tag="A")
                            aeng = nc.gpsimd if (k % 8) < gpsa else nc.vector
                            aeng.tensor_scalar(
                                at[:].bitcast(u16),
                                j2[:],
                                av[:, c : c + 1],
                                av[:, c + 1 : c + 2],
                                OP.is_equal,
                                OP.mult,
                            )
                            pre[(r, k)] = at

                for r in range(RPC):
                    ktrr = ktr[r]
                    ptail = nps[r] - (ktrr - 1) * 128
                    # E row: one big contiguous DMA + partial tail DMA; only
                    # used positions are transferred, matmuls slice K to match.
                    if noe:
                        et = e_const
                    else:
                        # one full-width DMA; the tail partitions beyond
                        # nps[r] carry padding rows whose A entries are all
                        # zero, so contracting them is harmless
                        et = epool.tile([128, KT * D], f8, tag="E")
                        for h in range(ehalves):
                            c0 = h * ktrr * D // ehalves
                            c1 = (h + 1) * ktrr * D // ehalves
                            if c1 > c0:
                                nc.sync.dma_start(et[:, c0:c1], E_in[r, :, c0:c1])

                    # one-hot A tiles, one fused DVE op per k-group, written
                    # through a u16 view for the 4x DVE mode
                    ak = []
                    for k in range(ktrr):
                        if aship:
                            gg = r * KT + k
                            ak.append(abig[:, gg * 128 : (gg + 1) * 128])
                            continue
                        if prebuild:
                            ak.append(pre[(r, k)])
                            continue
                        c = (r * KT + k) * 2
                        at = apool.tile([128, 128], f8, tag="A")
                        aeng = nc.gpsimd if (k % 8) < gpsa else nc.vector
                        aeng.tensor_scalar(
                            at[:].bitcast(u16),
                            j2[:],
                            av[:, c : c + 1],
                            av[:, c + 1 : c + 2],
                            OP.is_equal,
                            OP.mult,
                        )
                        ak.append(at)

                    if nomm:
                        continue

                    osb = opool.tile([128, ow[r]], i8, tag="osb")
                    g = 2 if drmerge is True else (int(drmerge) or 1)
                    groups = [
                        tuple(range(t, min(t + g, ktrr)))
                        for t in range(0, ktrr, g)
                    ]
                    for ks in groups:
                        # PSUM tiles are whole 2KB banks so every pool slot
                        # stays bank-aligned; pieces crossing a 512-f32 bank
                        # line are split below. Merged pairs pack two
                        # k-groups contiguously and drain in one op.
                        width = sum(NCHUNK * nmax[r][k] for k in ks)
                        assert width <= 512 * len(ks)
                        ps = pspool.tile([128, 512 * len(ks)], f32, tag="ps")
                        off = 0
                        for k in ks:
                            nm = nmax[r][k]
                            kp = 128
                            for c in range(NCHUNK):
                                lhsT = et[
                                    :kp, k * D + c * 128 : k * D + (c + 1) * 128
                                ]
                                p0 = off + c * nm
                                p1 = off + (c + 1) * nm
                                cuts = [p0] + [
                                    x for x in (512, 1024, 1536) if p0 < x < p1
                                ] + [p1]
                                for a, bnd in zip(cuts[:-1], cuts[1:]):
                                    nc.tensor.matmul(
                                        ps[:, a:bnd],
                                        lhsT,
                                        ak[k][:kp, a - p0 : bnd - p0],
                                        start=True,
                                        stop=True,
                                    )
                            off += NCHUNK * nm
                        # drain PSUM -> int8 (x2 undoes the /2 in the A scale)
                        src = ps[:, :width]
                        dst = osb[:, sbase[r][ks[0]] : sbase[r][ks[0]] + width]
                        if drpat == "alt":
                            use_dve = (ks[0] * dvek) % 8 >= 8 - dvek
                        else:
                            use_dve = ks[0] >= ktrr - dvek
                        if use_dve:
                            nc.vector.tensor_scalar(dst, src, 2.0, None, OP.mult)
                        else:
                            nc.scalar.activation(dst, src, AF.Copy, scale=2.0)

                    if not noout:
                        oeng = {"scalar": nc.scalar, "sync": nc.sync,
                                "gpsimd": nc.gpsimd, "vector": nc.vector}[outeng]
                        OW = ow[r] // ohalves
                        for h in range(ohalves):
                            oeng.dma_start(
                                out[r, :, h * OW : (h + 1) * OW],
                                osb[:, h * OW : (h + 1) * OW],
                            )
                    last_osb = osb

            if tok is not None:
                if last_osb is not None:
                    nc.sync.dma_start(tok[:], last_osb[:, :64].bitcast(f32))
                elif av is not None:
                    nc.sync.dma_start(tok[:], av[:, :16])
                else:
                    nc.sync.dma_start(tok[:], abig[:, :64].bitcast(f32))

    nc.compile()
    return nc


def _prep(bert_embedding, x_bert_offset, x_mask):
    st = x_bert_offset[..., 0].astype(np.int64)
    ed = x_bert_offset[..., 1].astype(np.int64)
    length = ed - st
    valid = (x_mask > 0) & (length > 0)
    len_c = np.maximum(length, 1).astype(np.float64)
    scale = np.where(valid, 127.0 / (QCLIP * np.sqrt(len_c)), 0.0).astype(np.float32)
    half8 = (scale / 2).astype(ml_dtypes.float8_e3m4)
    half8_bits = half8.view(np.uint8)
    scale8x = 2.0 * half8.astype(np.float32)  # exact dequant divisor

    # word index of each position (-1 if uncovered by a VALID word)
    st_ext = np.concatenate([st, ed[:, -1:]], axis=1)  # [B, W+1]
    word_of = np.full((B, S), -1, dtype=np.int64)
    s_idx = np.arange(S)
    for b in range(B):
        j = np.searchsorted(st_ext[b], s_idx, side="right") - 1
        ok = (j >= 0) & (j < W)
        wo = np.where(ok, j, -1)
        cov = (wo >= 0) & valid[b, np.clip(wo, 0, W - 1)]
        word_of[b] = np.where(cov, wo, -1)

    # compact away uncovered positions; per row-slot the count is the max
    # over cores (shared SPMD program). Rows with similar used-position
    # counts are grouped into the same slot so the max-over-cores padding
    # (shared program sizes) stays tight.
    perms = [np.nonzero(word_of[b] >= 0)[0] for b in range(B)]
    order = np.argsort([-len(p) for p in perms], kind="stable")
    assign = [
        [int(order[r * NCORES + c]) for r in range(RPC)] for c in range(NCORES)
    ]
    slot_of = np.zeros(B, dtype=np.int64)
    for c in range(NCORES):
        for r in range(RPC):
            slot_of[assign[c][r]] = r
    nps = [
        max(1, max(len(perms[assign[c][r]]) for c in range(NCORES)))
        for r in range(RPC)
    ]
    ktr = [(n + 127) // 128 for n in nps]

    # per (b, k): the ordered distinct words drawing from k-tile k
    words_bk = [[None] * KT for _ in range(B)]
    n_bk = np.zeros((B, KT), dtype=np.int64)
    for b in range(B):
        pb = perms[b]
        r = int(slot_of[b])
        for k in range(ktr[r]):
            seg = pb[k * 128 : (k + 1) * 128]
            wseg = word_of[b, seg] if len(seg) else np.empty(0, np.int64)
            wk = np.unique(wseg)  # sorted == position order (monotone)
            words_bk[b][k] = wk
            n_bk[b, k] = len(wk)

    nmax = [
        [
            int(max(n_bk[assign[c][r], k] for c in range(NCORES)))
            for k in range(ktr[r])
        ]
        for r in range(RPC)
    ]

    # E in partition-major e3m4 layout over compacted positions
    E = np.ascontiguousarray(bert_embedding, dtype=np.float32)
    E_h = np.zeros((B, 128, KT * D), dtype=ml_dtypes.float8_e3m4)
    for b in range(B):
        pb = perms[b]
        perm_pad = np.zeros(KT * 128, dtype=np.int64)
        perm_pad[: len(pb)] = pb
        E_h[b] = (
            E[b][perm_pad]
            .reshape(KT, 128, D)
            .transpose(1, 0, 2)
            .reshape(128, KT * D)
            .astype(ml_dtypes.float8_e3m4)
        )

    in_maps = []
    for c in range(NCORES):
        av = np.zeros((128, RPC * KT * 2), dtype=np.float32)
        abig = np.zeros((128, RPC * KT * 128), dtype=np.uint8)
        for r in range(RPC):
            b = assign[c][r]
            pb = perms[b]
            for k in range(ktr[r]):
                seg = pb[k * 128 : (k + 1) * 128]
                col = (r * KT + k) * 2
                if len(seg) == 0:
                    av[:, col] = -1.0
                    continue
                wseg = word_of[b, seg]
                slot = np.searchsorted(words_bk[b][k], wseg)
                bits = half8_bits[b, wseg].astype(np.int64)
                vals = bits << (8 * (slot & 1))
                idx = np.full(128, -1.0, dtype=np.float32)
                val = np.zeros(128, dtype=np.float32)
                idx[: len(seg)] = (slot >> 1).astype(np.float32)
                val[: len(seg)] = vals.astype(np.float32)
                av[:, col] = idx
                av[:, col + 1] = val
                abig[
                    np.arange(len(seg)), (r * KT + k) * 128 + slot
                ] = half8_bits[b, wseg]
        in_maps.append(
            {
                "E_in": np.stack([E_h[assign[c][r]] for r in range(RPC)]),
                "av_in": av,
                "A_in": abig.view(ml_dtypes.float8_e3m4),
            }
        )
    pairs = (tuple(nps), tuple(tuple(x) for x in nmax))
    unpack = (words_bk, scale8x, len_c, valid, ktr, nmax, assign)
    return pairs, in_maps, unpack


def kernel(bert_embedding, x_bert_offset, x_mask):
    from concourse.bass_utils import run_bass_kernel_spmd

    bert_embedding = np.asarray(bert_embedding, dtype=np.float32)
    x_bert_offset = np.asarray(x_bert_offset)
    x_mask = np.asarray(x_mask)
    pairs, in_maps, unpack = _prep(bert_embedding, x_bert_offset, x_mask)
    words_bk, scale8x, len_c, valid, ktr, nmax, assign = unpack
    key = repr(pairs)
    nc = _CACHE.get(key)
    if nc is None:
        nc = build_program(pairs)
        _CACHE[key] = nc
    res = run_bass_kernel_spmd(nc, in_maps, list(range(NCORES)))

    out = np.zeros((B, W, D), dtype=np.float32)
    for c in range(NCORES):
        dev = np.asarray(res.results[c]["out"])
        for r in range(RPC):
            b = assign[c][r]
            acc = np.zeros((W, D), dtype=np.float32)
            base = 0
            for k in range(ktr[r]):
                nm = nmax[r][k]
                wk = words_bk[b][k]
                n = len(wk)
                if n:
                    blk = (
                        dev[r, :, base : base + NCHUNK * nm]
                        .reshape(128, NCHUNK, nm)
                        .transpose(2, 1, 0)
                        .reshape(nm, D)
                        .astype(np.float32)
                    )
                    acc[wk] += blk[:n] / scale8x[b, wk, None]
                base += NCHUNK * nm
            mean = acc / len_c[b][:, None].astype(np.float32)
            out[b] = np.where(valid[b][:, None], mean, 0.0)
    return out
